# revision 8
# baseline (speedup 1.0000x reference)
"""CrossAttentionBlock Trainium2 kernel (v3: skewed pipeline, tuned head).

Shapes (hardcoded): x (16, 512, 64, 64) f32, context (16, 77, 768) f32.
Sharding: data-parallel over batch B=16 across 8 cores (2 batches/core).
Each core runs the full block on its 2 batches; weights replicated,
outputs gathered on host. No collectives.

v3 layout/schedule:
- Weights are pre-arranged on host to [128, ko, out] so their DMAs use
  contiguous 4KB-per-partition descriptors (v2's rearrange emitted ~2.5k
  1KB descriptors that delayed the ctx DMA by ~20us).
- Emission order: ctx DMAs -> layernorm -> weights -> ctx-side matmuls ->
  batch-0 x stream (stats per 2048-px half, residual copy on gpsimd) ->
  combine -> pipeline. Batch-1's x stream is spread across the first
  pipeline iterations to keep the in-order vector queue from blocking.
- 16 (batch, chunk) units flow through a 3-stage skewed pipeline
  (A: groupnorm-apply+q-proj two ahead, B: attention, C: out-proj) so the
  in-order PE queue always has ready work and stays at full p-state.

Softmax is unnormalized (logits small, no max subtraction); the
denominator is formed partition-replicated by an all-ones stationary
matmul and divided out during AV psum evacuation.
"""

import numpy as np
import ml_dtypes

import concourse.bass as bass
import concourse.tile as tile
from concourse import bacc
from concourse import mybir
from concourse.bass_utils import run_bass_kernel_spmd

F32 = mybir.dt.float32
BF16 = mybir.dt.bfloat16
AF = mybir.ActivationFunctionType
ALU = mybir.AluOpType

B, C, H, W = 16, 512, 64, 64
HW = H * W
S, CTX = 77, 768
HEADS = 8
HD = C // HEADS  # 64
GROUPS = 32
EPS = 1e-5
NCORES = 8
BPC = B // NCORES  # 2 batches per core
P = 128
NCH = HW // 512  # 8 pixel chunks of 512
KQ = C // P      # 4 chunks of 128 for C-contraction
KC = CTX // P    # 6 chunks for CTX-contraction
SCALE = HD ** (-0.5)
NU = BPC * NCH   # 16 pipeline units


def _ctx_ln(nc, pools, consts, b, ctxr, st):
    """DMA + LayerNorm(context) for batch b (vector work, no weights)."""
    small_pool = pools["small"]
    ctx_t = small_pool.tile([S, CTX], F32, tag=f"ctx{b}")
    nc.sync.dma_start(ctx_t, ctxr[b])
    lst = small_pool.tile([S, 3, 6], F32, tag=f"lst{b}")
    for i in range(3):
        nc.vector.bn_stats(lst[:, i, :], ctx_t[:, i * 256:(i + 1) * 256])
    lmv = small_pool.tile([S, 2], F32, tag=f"lmv{b}")
    nc.vector.bn_aggr(lmv, lst)
    nc.scalar.activation(lmv[:, 1:2], lmv[:, 1:2], AF.Sqrt,
                         bias=consts["eps77"], scale=1.0)
    nc.vector.reciprocal_approx_fast(out=lmv[:, 1:2], in_=lmv[:, 1:2])
    cn_t = small_pool.tile([S, CTX], F32, tag=f"cn{b}")
    nc.vector.tensor_scalar(cn_t, ctx_t, lmv[:, 0:1], lmv[:, 1:2],
                            ALU.subtract, ALU.mult)
    nc.vector.tensor_mul(cn_t, cn_t, consts["lnw_bc"])
    nc.vector.tensor_add(cn_t, cn_t, consts["lnb_bc"])
    st[b]["cn_t"] = cn_t


def _ctx_proj(nc, pools, consts, b, st):
    """cnT transpose -> kT, v_sc for batch b (PE work, needs weights)."""
    small_pool, ps_d, ps_av = pools["small"], pools["ps_d"], pools["ps_av"]
    cn_t = st[b].pop("cn_t")
    cnT = small_pool.tile([P, KC, S], BF16, tag=f"cnT{b}")
    for kc in range(KC):
        pt = ps_d.tile([P, S], F32, tag="pd")
        nc.tensor.transpose(pt, cn_t[:, kc * P:(kc + 1) * P],
                            consts["ident"][:S, :S])
        nc.vector.tensor_copy(cnT[:, kc, :], pt)

    kT = small_pool.tile([P, KQ, S], BF16, tag=f"kT{b}")
    for mo in range(KQ):
        pk = ps_av.tile([P, S], F32, tag="pav")
        for kc in range(KC):
            nc.tensor.matmul(pk, consts["kwT"][:, kc, mo * P:(mo + 1) * P],
                             cnT[:, kc, :], start=(kc == 0), stop=(kc == KC - 1))
        nc.scalar.activation(kT[:, mo, :], pk, AF.Identity,
                             bias=consts["kb"][:, mo:mo + 1], scale=1.0)

    pv = pools["ps_qk"].tile([S, C], F32, tag="pa")
    for kc in range(KC):
        nc.tensor.matmul(pv, cnT[:, kc, :], consts["vwT"][:, kc, :],
                         start=(kc == 0), stop=(kc == KC - 1))
    v_sc = small_pool.tile([S, C], BF16, tag=f"vsc{b}")
    nc.vector.tensor_add(v_sc, pv, consts["vb_bc"])

    st[b]["kT"] = kT
    st[b]["v_sc"] = v_sc


def _x_init(nc, pools, b, st):
    small_pool, xr_pool = pools["small"], pools["xr"]
    st[b]["gst"] = small_pool.tile([P, KQ, 8, 6], F32, tag=f"gst{b}",
                                   name=f"gst{b}")
    st[b]["mv_c"] = small_pool.tile([P, KQ, 2], F32, tag=f"mvc{b}",
                                    name=f"mvc{b}")
    st[b]["x_t"] = [xr_pool.tile([P, HW], BF16, tag=f"xr{b}_{co}",
                                 name=f"xr{b}_{co}")
                    for co in range(KQ)]


def _x_load_co(nc, pools, consts, b, co, xr, st):
    """Stream one 128-channel chunk of x for batch b: 2 half DMAs,
    bn_stats per 512 slice, gpsimd bias-folded bf16 residual copy."""
    x_pool = pools["x"]
    gst, mv_c, x_t = st[b]["gst"], st[b]["mv_c"], st[b]["x_t"]
    for h in range(2):
        hsl = slice(h * 2048, (h + 1) * 2048)
        xf = x_pool.tile([P, 2048], F32, tag="xf")
        nc.sync.dma_start(xf, xr[b, :, co, hsl])
        for sg in range(4):
            g = h * 4 + sg
            nc.vector.bn_stats(gst[:, co, g, :],
                               xf[:, sg * 512:(sg + 1) * 512])
        nc.gpsimd.tensor_scalar(x_t[co][:, hsl], xf,
                                consts["ob"][:, co:co + 1], None, ALU.add)
    nc.vector.bn_aggr(mv_c[:, co, :], gst[:, co])


def _stats_combine(nc, pools, consts, b, st):
    """Combine per-partition stats into per-group scale/shift."""
    small_pool, ps_d, ps_av = pools["small"], pools["ps_d"], pools["ps_av"]
    mv_c = st[b]["mv_c"]
    t3 = small_pool.tile([P, KQ, 3], F32, tag=f"t3{b}")
    nc.vector.tensor_copy(t3[:, :, 0:2], mv_c)
    nc.vector.tensor_mul(t3[:, :, 2:3], mv_c[:, :, 0:1], mv_c[:, :, 0:1])
    pg = ps_d.tile([GROUPS // KQ, KQ * 3], F32, tag="pd")
    nc.tensor.matmul(pg, consts["ind1"],
                     t3.rearrange("p a b -> p (a b)"), start=True, stop=True)
    g_sb = small_pool.tile([GROUPS // KQ, KQ, 3], F32, tag=f"gsb{b}")
    nc.vector.tensor_copy(g_sb.rearrange("p a b -> p (a b)"), pg)
    stats2 = small_pool.tile([GROUPS // KQ, 2, KQ], F32, tag=f"st2{b}")
    nc.vector.tensor_copy(stats2[:, 0, :], g_sb[:, :, 0])
    vt = small_pool.tile([GROUPS // KQ, KQ], F32, tag=f"vt{b}")
    nc.vector.tensor_add(vt, g_sb[:, :, 1], g_sb[:, :, 2])
    m2 = small_pool.tile([GROUPS // KQ, KQ], F32, tag=f"m2{b}")
    nc.vector.tensor_mul(m2, g_sb[:, :, 0], g_sb[:, :, 0])
    nc.vector.tensor_sub(vt, vt, m2)
    nc.scalar.activation(vt, vt, AF.Sqrt, bias=consts["eps8"], scale=1.0)
    nc.vector.reciprocal_approx_fast(out=stats2[:, 1, :], in_=vt)
    pbc = ps_av.tile([P, 2 * KQ], F32, tag="pav")
    nc.tensor.matmul(pbc, consts["ind2"],
                     stats2.rearrange("p a b -> p (a b)"), start=True, stop=True)
    sbc = small_pool.tile([P, 2, KQ], F32, tag=f"sbc{b}")
    nc.vector.tensor_copy(sbc.rearrange("p a b -> p (a b)"), pbc)
    scale_c = small_pool.tile([P, KQ], F32, tag=f"scl{b}")
    shift_c = small_pool.tile([P, KQ], F32, tag=f"shf{b}")
    nc.vector.tensor_mul(scale_c, sbc[:, 1, :], consts["gnw"])
    nc.vector.tensor_mul(shift_c, sbc[:, 0, :], scale_c)
    nc.vector.tensor_sub(shift_c, consts["gnb"], shift_c)
    shift2 = small_pool.tile([P, KQ], F32, tag=f"sh2{b}")
    nc.vector.tensor_mul(shift2, consts["ob"], scale_c)
    nc.vector.tensor_sub(shift2, shift_c, shift2)
    st[b]["scale_c"] = scale_c
    st[b]["shift2"] = shift2


def _stage_a_xnt(nc, pools, consts, st, u):
    """Groupnorm apply (vector) for unit u -> xnt."""
    b, n = divmod(u, NCH)
    nsl = slice(n * 512, (n + 1) * 512)
    x_t = st[b]["x_t"]
    scale_c, shift2 = st[b]["scale_c"], st[b]["shift2"]
    xnt = pools["xn"].tile([P, KQ, 512], BF16, tag="xnt")
    for kc in range(KQ):
        nc.vector.tensor_scalar(xnt[:, kc, :], x_t[kc][:, nsl],
                                scale_c[:, kc:kc + 1], shift2[:, kc:kc + 1],
                                ALU.mult, ALU.add)
    return xnt


def _stage_a_q(nc, pools, consts, st, u, xnt):
    """q projection (PE) for unit u -> qT(u)."""
    qT = pools["q"].tile([P, KQ, 512], BF16, tag="qT")
    for mo in range(KQ):
        pq = pools["ps_proj"].tile([P, 512], F32, tag="pmm")
        for kc in range(KQ):
            nc.tensor.matmul(pq, consts["qwT"][:, kc, mo * P:(mo + 1) * P],
                             xnt[:, kc, :], start=(kc == 0), stop=(kc == KQ - 1))
        nc.scalar.activation(qT[:, mo, :], pq, AF.Identity,
                             bias=consts["qb"][:, mo:mo + 1], scale=1.0)
    st["qT"][u] = qT


def _stage_a(nc, pools, consts, st, u):
    xnt = _stage_a_xnt(nc, pools, consts, st, u)
    _stage_a_q(nc, pools, consts, st, u, xnt)


def _stage_b(nc, pools, consts, st, u):
    """Attention for unit u: QK -> exp -> denom -> recip -> AV -> outT(u)."""
    b, n = divmod(u, NCH)
    kT, v_sc = st[b]["kT"], st[b]["v_sc"]
    qT = st["qT"].pop(u)
    ex_l = []
    for co in range(KQ):
        pa = pools["ps_qk"].tile([S, 2, 512], F32, tag="pa")
        nc.tensor.matmul(pa[:, 0, :], kT[0:HD, co, :], qT[0:HD, co, :],
                         start=True, stop=True, tile_position=(0, 0))
        nc.tensor.matmul(pa[:, 1, :], kT[HD:P, co, :], qT[HD:P, co, :],
                         start=True, stop=True, tile_position=(64, 0))
        ex = pools["exp"].tile([S, 2, 512], BF16, tag="ex")
        nc.scalar.activation(ex, pa, AF.Exp, scale=SCALE)
        ex_l.append(ex)
    outT = pools["o"].tile([P, KQ, 512], BF16, tag="outT")
    for co in range(KQ):
        ex = ex_l[co]
        pd = pools["ps_d"].tile([P, 512], F32, tag="pd")
        nc.tensor.matmul(pd[0:HD, :], consts["ones77"], ex[:, 0, :],
                         start=True, stop=True, tile_position=(0, 0))
        nc.tensor.matmul(pd[HD:P, :], consts["ones77"], ex[:, 1, :],
                         start=True, stop=True, tile_position=(0, 64))
        rc = pools["rc"].tile([P, 512], F32, tag="rc")
        nc.vector.reciprocal_approx_fast(out=rc, in_=pd)
        pav = pools["ps_av"].tile([P, 512], F32, tag="pav")
        h0, h1 = 2 * co, 2 * co + 1
        nc.tensor.matmul(pav[0:HD, :], v_sc[:, h0 * HD:(h0 + 1) * HD],
                         ex[:, 0, :], start=True, stop=True,
                         tile_position=(0, 0))
        nc.tensor.matmul(pav[HD:P, :], v_sc[:, h1 * HD:(h1 + 1) * HD],
                         ex[:, 1, :], start=True, stop=True,
                         tile_position=(0, 64))
        nc.vector.tensor_mul(outT[:, co, :], pav, rc)
    st["outT"][u] = outT


def _stage_c(nc, pools, consts, st, u, outr):
    """Out projection + residual + store for unit u."""
    b, n = divmod(u, NCH)
    nsl = slice(n * 512, (n + 1) * 512)
    x_t = st[b]["x_t"]
    outT = st["outT"].pop(u)
    for mo in range(KQ):
        po = pools["ps_proj"].tile([P, 512], F32, tag="pmm")
        for kc in range(KQ):
            nc.tensor.matmul(po, consts["owT"][:, kc, mo * P:(mo + 1) * P],
                             outT[:, kc, :], start=(kc == 0),
                             stop=(kc == KQ - 1))
        fin = pools["fin"].tile([P, 512], F32, tag="fin")
        nc.vector.tensor_add(fin, po, x_t[mo][:, nsl])
        nc.sync.dma_start(outr[b, :, mo, nsl], fin)


def build_nc(reps=1, loop_reps=0):
    nc = bacc.Bacc()

    x = nc.dram_tensor("x", [BPC, C, HW], F32, kind="ExternalInput")
    ctx_in = nc.dram_tensor("ctx", [BPC, S, CTX], F32, kind="ExternalInput")
    # weights pre-arranged host-side: [kp, ko, out]
    qwT = nc.dram_tensor("qwT", [P, KQ, C], BF16, kind="ExternalInput")
    kwT = nc.dram_tensor("kwT", [P, KC, C], BF16, kind="ExternalInput")
    vwT = nc.dram_tensor("vwT", [P, KC, C], BF16, kind="ExternalInput")
    owT = nc.dram_tensor("owT", [P, KQ, C], BF16, kind="ExternalInput")
    qb = nc.dram_tensor("qb", [C], F32, kind="ExternalInput")
    kb = nc.dram_tensor("kb", [C], F32, kind="ExternalInput")
    vb = nc.dram_tensor("vb", [C], F32, kind="ExternalInput")
    ob = nc.dram_tensor("ob", [C], F32, kind="ExternalInput")
    gnw = nc.dram_tensor("gnw", [C], F32, kind="ExternalInput")
    gnb = nc.dram_tensor("gnb", [C], F32, kind="ExternalInput")
    lnw = nc.dram_tensor("lnw", [CTX], F32, kind="ExternalInput")
    lnb = nc.dram_tensor("lnb", [CTX], F32, kind="ExternalInput")
    ident = nc.dram_tensor("ident", [P, P], F32, kind="ExternalInput")
    ones77 = nc.dram_tensor("ones77", [S, HD], BF16, kind="ExternalInput")
    ind1 = nc.dram_tensor("ind1", [P, GROUPS // KQ], F32, kind="ExternalInput")
    ind2 = nc.dram_tensor("ind2", [GROUPS // KQ, P], F32, kind="ExternalInput")
    out = nc.dram_tensor("out", [BPC, C, HW], F32, kind="ExternalOutput")

    xr = x[:].rearrange("b (co p) hw -> b p co hw", p=P)
    ctxr = ctx_in[:]
    outr = out[:].rearrange("b (co p) hw -> b p co hw", p=P)

    with tile.TileContext(nc) as tc:
        with (
            tc.tile_pool(name="singles", bufs=1) as singles,
            tc.tile_pool(name="xp", bufs=3) as x_pool,
            tc.tile_pool(name="xr", bufs=1) as xr_pool,
            tc.tile_pool(name="xnp", bufs=3) as xn_pool,
            tc.tile_pool(name="small", bufs=1) as small_pool,
            tc.tile_pool(name="qp", bufs=3) as q_pool,
            tc.tile_pool(name="op", bufs=2) as o_pool,
            tc.tile_pool(name="expp", bufs=5) as exp_pool,
            tc.tile_pool(name="rcp", bufs=2) as rc_pool,
            tc.tile_pool(name="finp", bufs=4) as fin_pool,
            tc.tile_pool(name="ps_proj", bufs=2, space="PSUM") as ps_proj,
            tc.tile_pool(name="ps_qk", bufs=2, space="PSUM") as ps_qk,
            tc.tile_pool(name="ps_d", bufs=1, space="PSUM") as ps_d,
            tc.tile_pool(name="ps_av", bufs=1, space="PSUM") as ps_av,
        ):
            pools = {
                "x": x_pool, "xr": xr_pool, "xn": xn_pool, "small": small_pool,
                "q": q_pool, "o": o_pool, "exp": exp_pool, "rc": rc_pool,
                "fin": fin_pool, "ps_proj": ps_proj, "ps_qk": ps_qk,
                "ps_d": ps_d, "ps_av": ps_av,
            }
            consts = {}

            def _small_consts():
                for name, src in (("qb", qb), ("kb", kb), ("ob", ob),
                                  ("gnw", gnw), ("gnb", gnb)):
                    t = singles.tile([P, KQ], F32, tag=name)
                    nc.sync.dma_start(t, src[:].rearrange("(a p) -> p a", p=P))
                    consts[name] = t
                t = singles.tile([P, P], F32, tag="ident")
                nc.sync.dma_start(t, ident[:])
                consts["ident"] = t
                t = singles.tile([S, HD], BF16, tag="ones77")
                nc.sync.dma_start(t, ones77[:])
                consts["ones77"] = t
                t = singles.tile([P, GROUPS // KQ], F32, tag="ind1")
                nc.sync.dma_start(t, ind1[:])
                consts["ind1"] = t
                t = singles.tile([GROUPS // KQ, P], F32, tag="ind2")
                nc.sync.dma_start(t, ind2[:])
                consts["ind2"] = t
                t = singles.tile([S, C], F32, tag="vb_bc")
                nc.gpsimd.dma_start(out=t, in_=vb[None, :].to_broadcast([S, C]))
                consts["vb_bc"] = t
                for name, src in (("lnw_bc", lnw), ("lnb_bc", lnb)):
                    t = singles.tile([S, CTX], F32, tag=name)
                    nc.gpsimd.dma_start(out=t,
                                        in_=src[None, :].to_broadcast([S, CTX]))
                    consts[name] = t
                t = singles.tile([S, 1], F32, tag="eps77")
                nc.vector.memset(t, EPS)
                consts["eps77"] = t
                t = singles.tile([GROUPS // KQ, 1], F32, tag="eps8")
                nc.vector.memset(t, EPS)
                consts["eps8"] = t

            def _weights():
                for name, src, ko in (("kwT", kwT, KC), ("vwT", vwT, KC),
                                      ("qwT", qwT, KQ), ("owT", owT, KQ)):
                    t = singles.tile([P, ko, C], BF16, tag=name)
                    nc.sync.dma_start(t, src[:])
                    consts[name] = t

            def build_once():
                st = {0: {}, 1: {}, "qT": {}, "outT": {}}
                # tiny DMAs first so ctx + consts land before the x flood
                _small_consts()
                for b in range(BPC):
                    _ctx_ln(nc, pools, consts, b, ctxr, st)
                _weights()
                # batch-0 x stream (DMA emission before ctx-side PE work so
                # the rings prioritize it; the PE work depends only on ctx)
                _x_init(nc, pools, 0, st)
                for co in range(KQ):
                    _x_load_co(nc, pools, consts, 0, co, xr, st)
                for b in range(BPC):
                    _ctx_proj(nc, pools, consts, b, st)
                _stats_combine(nc, pools, consts, 0, st)
                _x_init(nc, pools, 1, st)
                # skewed pipeline: A two ahead, C at current; batch-1 x
                # stream spread over the first iterations
                _stage_a(nc, pools, consts, st, 0)
                _stage_a(nc, pools, consts, st, 1)
                for u in range(NU):
                    xnt_next = (_stage_a_xnt(nc, pools, consts, st, u + 2)
                                if u + 2 < NU else None)
                    _stage_b(nc, pools, consts, st, u)
                    if xnt_next is not None:
                        _stage_a_q(nc, pools, consts, st, u + 2, xnt_next)
                    if u < KQ:
                        _x_load_co(nc, pools, consts, 1, u, xr, st)
                    elif u == KQ:
                        _stats_combine(nc, pools, consts, 1, st)
                    _stage_c(nc, pools, consts, st, u, outr)

            if loop_reps:
                with tc.For_i(0, loop_reps, 1):
                    build_once()
            else:
                for _rep in range(reps):
                    build_once()

    nc.finalize()
    return nc


_NC_CACHE = None


def _get_nc():
    global _NC_CACHE
    if _NC_CACHE is None:
        _NC_CACHE = build_nc()
    return _NC_CACHE


def _host_consts():
    bf = ml_dtypes.bfloat16
    g = GROUPS // KQ  # 8 groups per 128-channel chunk
    ind1 = np.zeros((P, g), np.float32)
    for p in range(P):
        ind1[p, p // 16] = 1.0 / 16.0
    ind2 = np.zeros((g, P), np.float32)
    for p in range(P):
        ind2[p // 16, p] = 1.0
    return {
        "ident": np.eye(P, dtype=np.float32),
        "ones77": np.ones((S, HD), dtype=bf),
        "ind1": ind1,
        "ind2": ind2,
    }


def _w_arrange(w, ko):
    """[out, in] weight -> [kp, ko, out] bf16 with contiguous 4KB lines."""
    bf = ml_dtypes.bfloat16
    wT = np.ascontiguousarray(np.asarray(w, np.float32).T)  # [in, out]
    return np.ascontiguousarray(
        wT.reshape(ko, P, wT.shape[1]).transpose(1, 0, 2)).astype(bf)


def make_in_maps(x, context, gn_w, gn_b, ln_w, ln_b, q_w, q_b, k_w, k_b,
                 v_w, v_b, out_w, out_b):
    x = np.asarray(x, np.float32).reshape(B, C, HW)
    context = np.ascontiguousarray(np.asarray(context, np.float32))
    shared = {
        "qwT": _w_arrange(q_w, KQ),
        "kwT": _w_arrange(k_w, KC),
        "vwT": _w_arrange(v_w, KC),
        "owT": _w_arrange(out_w, KQ),
        "qb": np.asarray(q_b, np.float32),
        "kb": np.asarray(k_b, np.float32),
        "vb": np.asarray(v_b, np.float32),
        "ob": np.asarray(out_b, np.float32),
        "gnw": np.asarray(gn_w, np.float32),
        "gnb": np.asarray(gn_b, np.float32),
        "lnw": np.asarray(ln_w, np.float32),
        "lnb": np.asarray(ln_b, np.float32),
        **_host_consts(),
    }
    in_maps = []
    for i in range(NCORES):
        m = dict(shared)
        m["x"] = np.ascontiguousarray(x[i * BPC:(i + 1) * BPC])
        m["ctx"] = np.ascontiguousarray(context[i * BPC:(i + 1) * BPC])
        in_maps.append(m)
    return in_maps


def kernel(x, context, gn_w, gn_b, ln_w, ln_b, q_w, q_b, k_w, k_b,
           v_w, v_b, out_w, out_b):
    in_maps = make_in_maps(x, context, gn_w, gn_b, ln_w, ln_b, q_w, q_b,
                           k_w, k_b, v_w, v_b, out_w, out_b)
    nc = _get_nc()
    res = run_bass_kernel_spmd(nc, in_maps, core_ids=list(range(NCORES)))
    outs = [r["out"] for r in res.results]
    return np.concatenate(outs, axis=0).reshape(B, C, H, W)


if __name__ == "__main__":
    rng = np.random.default_rng(0)
    inputs = {
        "x": rng.standard_normal((B, C, H, W), np.float32),
        "context": rng.standard_normal((B, S, CTX), np.float32),
        "gn_w": np.ones(C, np.float32), "gn_b": np.zeros(C, np.float32),
        "ln_w": np.ones(CTX, np.float32), "ln_b": np.zeros(CTX, np.float32),
        "q_w": rng.standard_normal((C, C), np.float32) * 0.02,
        "q_b": np.zeros(C, np.float32),
        "k_w": rng.standard_normal((C, CTX), np.float32) * 0.02,
        "k_b": np.zeros(C, np.float32),
        "v_w": rng.standard_normal((C, CTX), np.float32) * 0.02,
        "v_b": np.zeros(C, np.float32),
        "out_w": rng.standard_normal((C, C), np.float32) * 0.02,
        "out_b": np.zeros(C, np.float32),
    }
    out = kernel(**inputs)
    print(out.shape, out.dtype)


# revision 9
# speedup vs baseline: 2.7338x; 2.7338x over previous
"""CrossAttentionBlock Trainium2 kernel (v3: skewed pipeline, tuned head).

Shapes (hardcoded): x (16, 512, 64, 64) f32, context (16, 77, 768) f32.
Sharding: data-parallel over batch B=16 across 8 cores (2 batches/core).
Each core runs the full block on its 2 batches; weights replicated,
outputs gathered on host. No collectives.

v3 layout/schedule:
- Weights are pre-arranged on host to [128, ko, out] so their DMAs use
  contiguous 4KB-per-partition descriptors (v2's rearrange emitted ~2.5k
  1KB descriptors that delayed the ctx DMA by ~20us).
- Emission order: ctx DMAs -> layernorm -> weights -> ctx-side matmuls ->
  batch-0 x stream (stats per 2048-px half, residual copy on gpsimd) ->
  combine -> pipeline. Batch-1's x stream is spread across the first
  pipeline iterations to keep the in-order vector queue from blocking.
- 16 (batch, chunk) units flow through a 3-stage skewed pipeline
  (A: groupnorm-apply+q-proj two ahead, B: attention, C: out-proj) so the
  in-order PE queue always has ready work and stays at full p-state.

Softmax is unnormalized (logits small, no max subtraction); the
denominator is formed partition-replicated by an all-ones stationary
matmul and divided out during AV psum evacuation.
"""

import numpy as np
import ml_dtypes

import concourse.bass as bass
import concourse.tile as tile
from concourse import bacc
from concourse import mybir
from concourse.bass_utils import run_bass_kernel_spmd

F32 = mybir.dt.float32
BF16 = mybir.dt.bfloat16
AF = mybir.ActivationFunctionType
ALU = mybir.AluOpType

B, C, H, W = 16, 512, 64, 64
HW = H * W
S, CTX = 77, 768
HEADS = 8
HD = C // HEADS  # 64
GROUPS = 32
EPS = 1e-5
NCORES = 8
BPC = B // NCORES  # 2 batches per core
P = 128
NCH = HW // 512  # 8 pixel chunks of 512
KQ = C // P      # 4 chunks of 128 for C-contraction
KC = CTX // P    # 6 chunks for CTX-contraction
SCALE = HD ** (-0.5)
NU = BPC * NCH   # 16 pipeline units


def _ctx_ln(nc, pools, consts, b, ctxr, st):
    """DMA + LayerNorm(context) for batch b (vector work, no weights)."""
    small_pool = pools["small"]
    ctx_t = small_pool.tile([S, CTX], F32, tag=f"ctx{b}")
    nc.sync.dma_start(ctx_t, ctxr[b])
    lst = small_pool.tile([S, 3, 6], F32, tag=f"lst{b}")
    for i in range(3):
        nc.vector.bn_stats(lst[:, i, :], ctx_t[:, i * 256:(i + 1) * 256])
    lmv = small_pool.tile([S, 2], F32, tag=f"lmv{b}")
    nc.vector.bn_aggr(lmv, lst)
    nc.scalar.activation(lmv[:, 1:2], lmv[:, 1:2], AF.Sqrt,
                         bias=consts["eps77"], scale=1.0)
    nc.vector.reciprocal_approx_fast(out=lmv[:, 1:2], in_=lmv[:, 1:2])
    cn_t = small_pool.tile([S, CTX], F32, tag=f"cn{b}")
    nc.vector.tensor_scalar(cn_t, ctx_t, lmv[:, 0:1], lmv[:, 1:2],
                            ALU.subtract, ALU.mult)
    nc.vector.tensor_mul(cn_t, cn_t, consts["lnw_bc"])
    nc.vector.tensor_add(cn_t, cn_t, consts["lnb_bc"])
    st[b]["cn_t"] = cn_t


def _ctx_proj(nc, pools, consts, b, st):
    """cnT transpose -> kT, v_sc for batch b (PE work, needs weights)."""
    small_pool, ps_d, ps_av = pools["small"], pools["ps_d"], pools["ps_av"]
    cn_t = st[b].pop("cn_t")
    cnT = small_pool.tile([P, KC, S], BF16, tag=f"cnT{b}")
    for kc in range(KC):
        pt = ps_d.tile([P, S], F32, tag="pd")
        nc.tensor.transpose(pt, cn_t[:, kc * P:(kc + 1) * P],
                            consts["ident"][:S, :S])
        nc.vector.tensor_copy(cnT[:, kc, :], pt)

    kT = small_pool.tile([P, KQ, S], BF16, tag=f"kT{b}")
    for mo in range(KQ):
        pk = ps_av.tile([P, S], F32, tag="pav")
        for kc in range(KC):
            nc.tensor.matmul(pk, consts["kwT"][:, kc, mo * P:(mo + 1) * P],
                             cnT[:, kc, :], start=(kc == 0), stop=(kc == KC - 1))
        nc.scalar.activation(kT[:, mo, :], pk, AF.Identity,
                             bias=consts["kb"][:, mo:mo + 1], scale=1.0)

    pv = pools["ps_qk"].tile([S, C], F32, tag="pa")
    for kc in range(KC):
        nc.tensor.matmul(pv, cnT[:, kc, :], consts["vwT"][:, kc, :],
                         start=(kc == 0), stop=(kc == KC - 1))
    v_sc = small_pool.tile([S, C], BF16, tag=f"vsc{b}")
    nc.vector.tensor_add(v_sc, pv, consts["vb_bc"])

    st[b]["kT"] = kT
    st[b]["v_sc"] = v_sc


def _x_init(nc, pools, b, st):
    small_pool, xr_pool = pools["small"], pools["xr"]
    st[b]["gst"] = small_pool.tile([P, KQ, 8, 6], F32, tag=f"gst{b}",
                                   name=f"gst{b}")
    st[b]["mv_c"] = small_pool.tile([P, KQ, 2], F32, tag=f"mvc{b}",
                                    name=f"mvc{b}")
    st[b]["x_t"] = [xr_pool.tile([P, HW], BF16, tag=f"xr{b}_{co}",
                                 name=f"xr{b}_{co}")
                    for co in range(KQ)]


def _x_load_co(nc, pools, consts, b, co, xr, st):
    """Stream one 128-channel chunk of x for batch b: 2 half DMAs,
    bn_stats per 512 slice, gpsimd bias-folded bf16 residual copy."""
    x_pool = pools["x"]
    gst, mv_c, x_t = st[b]["gst"], st[b]["mv_c"], st[b]["x_t"]
    for h in range(2):
        hsl = slice(h * 2048, (h + 1) * 2048)
        xf = x_pool.tile([P, 2048], F32, tag="xf")
        nc.sync.dma_start(xf, xr[b, :, co, hsl])
        for sg in range(4):
            g = h * 4 + sg
            nc.vector.bn_stats(gst[:, co, g, :],
                               xf[:, sg * 512:(sg + 1) * 512])
        nc.scalar.activation(x_t[co][:, hsl], xf, AF.Identity,
                             bias=consts["ob"][:, co:co + 1], scale=1.0)
    nc.vector.bn_aggr(mv_c[:, co, :], gst[:, co])


def _stats_combine(nc, pools, consts, b, st):
    """Combine per-partition stats into per-group scale/shift."""
    small_pool, ps_d, ps_av = pools["small"], pools["ps_d"], pools["ps_av"]
    mv_c = st[b]["mv_c"]
    t3 = small_pool.tile([P, KQ, 3], F32, tag=f"t3{b}")
    nc.vector.tensor_copy(t3[:, :, 0:2], mv_c)
    nc.vector.tensor_mul(t3[:, :, 2:3], mv_c[:, :, 0:1], mv_c[:, :, 0:1])
    pg = ps_d.tile([GROUPS // KQ, KQ * 3], F32, tag="pd")
    nc.tensor.matmul(pg, consts["ind1"],
                     t3.rearrange("p a b -> p (a b)"), start=True, stop=True)
    g_sb = small_pool.tile([GROUPS // KQ, KQ, 3], F32, tag=f"gsb{b}")
    nc.vector.tensor_copy(g_sb.rearrange("p a b -> p (a b)"), pg)
    stats2 = small_pool.tile([GROUPS // KQ, 2, KQ], F32, tag=f"st2{b}")
    nc.vector.tensor_copy(stats2[:, 0, :], g_sb[:, :, 0])
    vt = small_pool.tile([GROUPS // KQ, KQ], F32, tag=f"vt{b}")
    nc.vector.tensor_add(vt, g_sb[:, :, 1], g_sb[:, :, 2])
    m2 = small_pool.tile([GROUPS // KQ, KQ], F32, tag=f"m2{b}")
    nc.vector.tensor_mul(m2, g_sb[:, :, 0], g_sb[:, :, 0])
    nc.vector.tensor_sub(vt, vt, m2)
    nc.scalar.activation(vt, vt, AF.Sqrt, bias=consts["eps8"], scale=1.0)
    nc.vector.reciprocal_approx_fast(out=stats2[:, 1, :], in_=vt)
    pbc = ps_av.tile([P, 2 * KQ], F32, tag="pav")
    nc.tensor.matmul(pbc, consts["ind2"],
                     stats2.rearrange("p a b -> p (a b)"), start=True, stop=True)
    sbc = small_pool.tile([P, 2, KQ], F32, tag=f"sbc{b}")
    nc.vector.tensor_copy(sbc.rearrange("p a b -> p (a b)"), pbc)
    scale_c = small_pool.tile([P, KQ], F32, tag=f"scl{b}")
    shift_c = small_pool.tile([P, KQ], F32, tag=f"shf{b}")
    nc.vector.tensor_mul(scale_c, sbc[:, 1, :], consts["gnw"])
    nc.vector.tensor_mul(shift_c, sbc[:, 0, :], scale_c)
    nc.vector.tensor_sub(shift_c, consts["gnb"], shift_c)
    shift2 = small_pool.tile([P, KQ], F32, tag=f"sh2{b}")
    nc.vector.tensor_mul(shift2, consts["ob"], scale_c)
    nc.vector.tensor_sub(shift2, shift_c, shift2)
    st[b]["scale_c"] = scale_c
    st[b]["shift2"] = shift2


def _stage_a_xnt(nc, pools, consts, st, u):
    """Groupnorm apply (vector) for unit u -> xnt."""
    b, n = divmod(u, NCH)
    nsl = slice(n * 512, (n + 1) * 512)
    x_t = st[b]["x_t"]
    scale_c, shift2 = st[b]["scale_c"], st[b]["shift2"]
    xnt = pools["xn"].tile([P, KQ, 512], BF16, tag="xnt")
    for kc in range(KQ):
        nc.vector.tensor_scalar(xnt[:, kc, :], x_t[kc][:, nsl],
                                scale_c[:, kc:kc + 1], shift2[:, kc:kc + 1],
                                ALU.mult, ALU.add)
    return xnt


def _stage_a_q(nc, pools, consts, st, u, xnt):
    """q projection (PE) for unit u -> qT(u)."""
    qT = pools["q"].tile([P, KQ, 512], BF16, tag="qT")
    for mo in range(KQ):
        pq = pools["ps_proj"].tile([P, 512], F32, tag="pmm")
        for kc in range(KQ):
            nc.tensor.matmul(pq, consts["qwT"][:, kc, mo * P:(mo + 1) * P],
                             xnt[:, kc, :], start=(kc == 0), stop=(kc == KQ - 1))
        nc.scalar.activation(qT[:, mo, :], pq, AF.Identity,
                             bias=consts["qb"][:, mo:mo + 1], scale=1.0)
    st["qT"][u] = qT


def _stage_a(nc, pools, consts, st, u):
    xnt = _stage_a_xnt(nc, pools, consts, st, u)
    _stage_a_q(nc, pools, consts, st, u, xnt)


def _stage_b(nc, pools, consts, st, u):
    """Attention for unit u: QK -> exp -> denom -> recip -> AV -> outT(u)."""
    b, n = divmod(u, NCH)
    kT, v_sc = st[b]["kT"], st[b]["v_sc"]
    qT = st["qT"].pop(u)
    ex_l = []
    for co in range(KQ):
        pa = pools["ps_qk"].tile([S, 2, 512], F32, tag="pa")
        nc.tensor.matmul(pa[:, 0, :], kT[0:HD, co, :], qT[0:HD, co, :],
                         start=True, stop=True, tile_position=(0, 0))
        nc.tensor.matmul(pa[:, 1, :], kT[HD:P, co, :], qT[HD:P, co, :],
                         start=True, stop=True, tile_position=(64, 0))
        ex = pools["exp"].tile([S, 2, 512], BF16, tag="ex")
        nc.scalar.activation(ex, pa, AF.Exp, scale=SCALE)
        ex_l.append(ex)
    outT = pools["o"].tile([P, KQ, 512], BF16, tag="outT")
    for co in range(KQ):
        ex = ex_l[co]
        pd = pools["ps_d"].tile([P, 512], F32, tag="pd")
        nc.tensor.matmul(pd[0:HD, :], consts["ones77"], ex[:, 0, :],
                         start=True, stop=True, tile_position=(0, 0))
        nc.tensor.matmul(pd[HD:P, :], consts["ones77"], ex[:, 1, :],
                         start=True, stop=True, tile_position=(0, 64))
        rc = pools["rc"].tile([P, 512], F32, tag="rc")
        nc.vector.reciprocal_approx_fast(out=rc, in_=pd)
        pav = pools["ps_av"].tile([P, 512], F32, tag="pav")
        h0, h1 = 2 * co, 2 * co + 1
        nc.tensor.matmul(pav[0:HD, :], v_sc[:, h0 * HD:(h0 + 1) * HD],
                         ex[:, 0, :], start=True, stop=True,
                         tile_position=(0, 0))
        nc.tensor.matmul(pav[HD:P, :], v_sc[:, h1 * HD:(h1 + 1) * HD],
                         ex[:, 1, :], start=True, stop=True,
                         tile_position=(0, 64))
        nc.vector.tensor_mul(outT[:, co, :], pav, rc)
    st["outT"][u] = outT


def _stage_c(nc, pools, consts, st, u, outr):
    """Out projection + residual + store for unit u."""
    b, n = divmod(u, NCH)
    nsl = slice(n * 512, (n + 1) * 512)
    x_t = st[b]["x_t"]
    outT = st["outT"].pop(u)
    for mo in range(KQ):
        po = pools["ps_proj"].tile([P, 512], F32, tag="pmm")
        for kc in range(KQ):
            nc.tensor.matmul(po, consts["owT"][:, kc, mo * P:(mo + 1) * P],
                             outT[:, kc, :], start=(kc == 0),
                             stop=(kc == KQ - 1))
        fin = pools["fin"].tile([P, 512], F32, tag="fin")
        nc.vector.tensor_add(fin, po, x_t[mo][:, nsl])
        nc.sync.dma_start(outr[b, :, mo, nsl], fin)


def build_nc(reps=1, loop_reps=0):
    nc = bacc.Bacc()

    x = nc.dram_tensor("x", [BPC, C, HW], F32, kind="ExternalInput")
    ctx_in = nc.dram_tensor("ctx", [BPC, S, CTX], F32, kind="ExternalInput")
    # weights pre-arranged host-side: [kp, ko, out]
    qwT = nc.dram_tensor("qwT", [P, KQ, C], BF16, kind="ExternalInput")
    kwT = nc.dram_tensor("kwT", [P, KC, C], BF16, kind="ExternalInput")
    vwT = nc.dram_tensor("vwT", [P, KC, C], BF16, kind="ExternalInput")
    owT = nc.dram_tensor("owT", [P, KQ, C], BF16, kind="ExternalInput")
    qb = nc.dram_tensor("qb", [C], F32, kind="ExternalInput")
    kb = nc.dram_tensor("kb", [C], F32, kind="ExternalInput")
    vb = nc.dram_tensor("vb", [C], F32, kind="ExternalInput")
    ob = nc.dram_tensor("ob", [C], F32, kind="ExternalInput")
    gnw = nc.dram_tensor("gnw", [C], F32, kind="ExternalInput")
    gnb = nc.dram_tensor("gnb", [C], F32, kind="ExternalInput")
    lnw = nc.dram_tensor("lnw", [CTX], F32, kind="ExternalInput")
    lnb = nc.dram_tensor("lnb", [CTX], F32, kind="ExternalInput")
    ident = nc.dram_tensor("ident", [P, P], F32, kind="ExternalInput")
    ones77 = nc.dram_tensor("ones77", [S, HD], BF16, kind="ExternalInput")
    ind1 = nc.dram_tensor("ind1", [P, GROUPS // KQ], F32, kind="ExternalInput")
    ind2 = nc.dram_tensor("ind2", [GROUPS // KQ, P], F32, kind="ExternalInput")
    out = nc.dram_tensor("out", [BPC, C, HW], F32, kind="ExternalOutput")

    xr = x[:].rearrange("b (co p) hw -> b p co hw", p=P)
    ctxr = ctx_in[:]
    outr = out[:].rearrange("b (co p) hw -> b p co hw", p=P)

    with tile.TileContext(nc) as tc:
        with (
            tc.tile_pool(name="singles", bufs=1) as singles,
            tc.tile_pool(name="xp", bufs=3) as x_pool,
            tc.tile_pool(name="xr", bufs=1) as xr_pool,
            tc.tile_pool(name="xnp", bufs=3) as xn_pool,
            tc.tile_pool(name="small", bufs=1) as small_pool,
            tc.tile_pool(name="qp", bufs=3) as q_pool,
            tc.tile_pool(name="op", bufs=2) as o_pool,
            tc.tile_pool(name="expp", bufs=5) as exp_pool,
            tc.tile_pool(name="rcp", bufs=2) as rc_pool,
            tc.tile_pool(name="finp", bufs=4) as fin_pool,
            tc.tile_pool(name="ps_proj", bufs=2, space="PSUM") as ps_proj,
            tc.tile_pool(name="ps_qk", bufs=2, space="PSUM") as ps_qk,
            tc.tile_pool(name="ps_d", bufs=1, space="PSUM") as ps_d,
            tc.tile_pool(name="ps_av", bufs=1, space="PSUM") as ps_av,
        ):
            pools = {
                "x": x_pool, "xr": xr_pool, "xn": xn_pool, "small": small_pool,
                "q": q_pool, "o": o_pool, "exp": exp_pool, "rc": rc_pool,
                "fin": fin_pool, "ps_proj": ps_proj, "ps_qk": ps_qk,
                "ps_d": ps_d, "ps_av": ps_av,
            }
            consts = {}

            def _small_consts():
                for name, src in (("qb", qb), ("kb", kb), ("ob", ob),
                                  ("gnw", gnw), ("gnb", gnb)):
                    t = singles.tile([P, KQ], F32, tag=name)
                    nc.sync.dma_start(t, src[:].rearrange("(a p) -> p a", p=P))
                    consts[name] = t
                t = singles.tile([P, P], F32, tag="ident")
                nc.sync.dma_start(t, ident[:])
                consts["ident"] = t
                t = singles.tile([S, HD], BF16, tag="ones77")
                nc.sync.dma_start(t, ones77[:])
                consts["ones77"] = t
                t = singles.tile([P, GROUPS // KQ], F32, tag="ind1")
                nc.sync.dma_start(t, ind1[:])
                consts["ind1"] = t
                t = singles.tile([GROUPS // KQ, P], F32, tag="ind2")
                nc.sync.dma_start(t, ind2[:])
                consts["ind2"] = t
                t = singles.tile([S, C], F32, tag="vb_bc")
                nc.gpsimd.dma_start(out=t, in_=vb[None, :].to_broadcast([S, C]))
                consts["vb_bc"] = t
                for name, src in (("lnw_bc", lnw), ("lnb_bc", lnb)):
                    t = singles.tile([S, CTX], F32, tag=name)
                    nc.gpsimd.dma_start(out=t,
                                        in_=src[None, :].to_broadcast([S, CTX]))
                    consts[name] = t
                t = singles.tile([S, 1], F32, tag="eps77")
                nc.vector.memset(t, EPS)
                consts["eps77"] = t
                t = singles.tile([GROUPS // KQ, 1], F32, tag="eps8")
                nc.vector.memset(t, EPS)
                consts["eps8"] = t

            def _weights():
                for name, src, ko in (("kwT", kwT, KC), ("vwT", vwT, KC),
                                      ("qwT", qwT, KQ), ("owT", owT, KQ)):
                    t = singles.tile([P, ko, C], BF16, tag=name)
                    nc.sync.dma_start(t, src[:])
                    consts[name] = t

            def build_once():
                st = {0: {}, 1: {}, "qT": {}, "outT": {}}
                # tiny DMAs first so ctx + consts land before the x flood
                _small_consts()
                for b in range(BPC):
                    _ctx_ln(nc, pools, consts, b, ctxr, st)
                _weights()
                # batch-0 x stream (DMA emission before ctx-side PE work so
                # the rings prioritize it; the PE work depends only on ctx)
                _x_init(nc, pools, 0, st)
                for co in range(KQ):
                    _x_load_co(nc, pools, consts, 0, co, xr, st)
                for b in range(BPC):
                    _ctx_proj(nc, pools, consts, b, st)
                _stats_combine(nc, pools, consts, 0, st)
                _x_init(nc, pools, 1, st)
                # skewed pipeline: A two ahead, C at current; batch-1 x
                # stream spread over the first iterations
                _stage_a(nc, pools, consts, st, 0)
                _stage_a(nc, pools, consts, st, 1)
                for u in range(NU):
                    xnt_next = (_stage_a_xnt(nc, pools, consts, st, u + 2)
                                if u + 2 < NU else None)
                    _stage_b(nc, pools, consts, st, u)
                    if xnt_next is not None:
                        _stage_a_q(nc, pools, consts, st, u + 2, xnt_next)
                    if u < KQ:
                        _x_load_co(nc, pools, consts, 1, u, xr, st)
                    elif u == KQ:
                        _stats_combine(nc, pools, consts, 1, st)
                    _stage_c(nc, pools, consts, st, u, outr)

            if loop_reps:
                with tc.For_i(0, loop_reps, 1):
                    build_once()
            else:
                for _rep in range(reps):
                    build_once()

    nc.finalize()
    return nc


_NC_CACHE = None


def _get_nc():
    global _NC_CACHE
    if _NC_CACHE is None:
        _NC_CACHE = build_nc()
    return _NC_CACHE


def _host_consts():
    bf = ml_dtypes.bfloat16
    g = GROUPS // KQ  # 8 groups per 128-channel chunk
    ind1 = np.zeros((P, g), np.float32)
    for p in range(P):
        ind1[p, p // 16] = 1.0 / 16.0
    ind2 = np.zeros((g, P), np.float32)
    for p in range(P):
        ind2[p // 16, p] = 1.0
    return {
        "ident": np.eye(P, dtype=np.float32),
        "ones77": np.ones((S, HD), dtype=bf),
        "ind1": ind1,
        "ind2": ind2,
    }


def _w_arrange(w, ko):
    """[out, in] weight -> [kp, ko, out] bf16 with contiguous 4KB lines."""
    bf = ml_dtypes.bfloat16
    wT = np.ascontiguousarray(np.asarray(w, np.float32).T)  # [in, out]
    return np.ascontiguousarray(
        wT.reshape(ko, P, wT.shape[1]).transpose(1, 0, 2)).astype(bf)


def make_in_maps(x, context, gn_w, gn_b, ln_w, ln_b, q_w, q_b, k_w, k_b,
                 v_w, v_b, out_w, out_b):
    x = np.asarray(x, np.float32).reshape(B, C, HW)
    context = np.ascontiguousarray(np.asarray(context, np.float32))
    shared = {
        "qwT": _w_arrange(q_w, KQ),
        "kwT": _w_arrange(k_w, KC),
        "vwT": _w_arrange(v_w, KC),
        "owT": _w_arrange(out_w, KQ),
        "qb": np.asarray(q_b, np.float32),
        "kb": np.asarray(k_b, np.float32),
        "vb": np.asarray(v_b, np.float32),
        "ob": np.asarray(out_b, np.float32),
        "gnw": np.asarray(gn_w, np.float32),
        "gnb": np.asarray(gn_b, np.float32),
        "lnw": np.asarray(ln_w, np.float32),
        "lnb": np.asarray(ln_b, np.float32),
        **_host_consts(),
    }
    in_maps = []
    for i in range(NCORES):
        m = dict(shared)
        m["x"] = np.ascontiguousarray(x[i * BPC:(i + 1) * BPC])
        m["ctx"] = np.ascontiguousarray(context[i * BPC:(i + 1) * BPC])
        in_maps.append(m)
    return in_maps


def kernel(x, context, gn_w, gn_b, ln_w, ln_b, q_w, q_b, k_w, k_b,
           v_w, v_b, out_w, out_b):
    in_maps = make_in_maps(x, context, gn_w, gn_b, ln_w, ln_b, q_w, q_b,
                           k_w, k_b, v_w, v_b, out_w, out_b)
    nc = _get_nc()
    res = run_bass_kernel_spmd(nc, in_maps, core_ids=list(range(NCORES)))
    outs = [r["out"] for r in res.results]
    return np.concatenate(outs, axis=0).reshape(B, C, H, W)


if __name__ == "__main__":
    rng = np.random.default_rng(0)
    inputs = {
        "x": rng.standard_normal((B, C, H, W), np.float32),
        "context": rng.standard_normal((B, S, CTX), np.float32),
        "gn_w": np.ones(C, np.float32), "gn_b": np.zeros(C, np.float32),
        "ln_w": np.ones(CTX, np.float32), "ln_b": np.zeros(CTX, np.float32),
        "q_w": rng.standard_normal((C, C), np.float32) * 0.02,
        "q_b": np.zeros(C, np.float32),
        "k_w": rng.standard_normal((C, CTX), np.float32) * 0.02,
        "k_b": np.zeros(C, np.float32),
        "v_w": rng.standard_normal((C, CTX), np.float32) * 0.02,
        "v_b": np.zeros(C, np.float32),
        "out_w": rng.standard_normal((C, C), np.float32) * 0.02,
        "out_b": np.zeros(C, np.float32),
    }
    out = kernel(**inputs)
    print(out.shape, out.dtype)


# revision 22
# speedup vs baseline: 2.9006x; 1.0610x over previous
"""CrossAttentionBlock Trainium2 kernel (v3: skewed pipeline, tuned head).

Shapes (hardcoded): x (16, 512, 64, 64) f32, context (16, 77, 768) f32.
Sharding: data-parallel over batch B=16 across 8 cores (2 batches/core).
Each core runs the full block on its 2 batches; weights replicated,
outputs gathered on host. No collectives.

v3 layout/schedule:
- Weights are pre-arranged on host to [128, ko, out] so their DMAs use
  contiguous 4KB-per-partition descriptors (v2's rearrange emitted ~2.5k
  1KB descriptors that delayed the ctx DMA by ~20us).
- Emission order: ctx DMAs -> layernorm -> weights -> ctx-side matmuls ->
  batch-0 x stream (stats per 2048-px half, residual copy on gpsimd) ->
  combine -> pipeline. Batch-1's x stream is spread across the first
  pipeline iterations to keep the in-order vector queue from blocking.
- 16 (batch, chunk) units flow through a 3-stage skewed pipeline
  (A: groupnorm-apply+q-proj two ahead, B: attention, C: out-proj) so the
  in-order PE queue always has ready work and stays at full p-state.

Softmax is unnormalized (logits small, no max subtraction); the
denominator is formed partition-replicated by an all-ones stationary
matmul and divided out during AV psum evacuation.
"""

import numpy as np
import ml_dtypes

import concourse.bass as bass
import concourse.tile as tile
from concourse import bacc
from concourse import mybir
from concourse.bass_utils import run_bass_kernel_spmd

F32 = mybir.dt.float32
BF16 = mybir.dt.bfloat16
F8 = mybir.dt.float8e4
AF = mybir.ActivationFunctionType
ALU = mybir.AluOpType
PM = mybir.MatmulPerfMode

FP8_PROJ = True  # q/out projections via fp8e4 DoubleRow matmuls

B, C, H, W = 16, 512, 64, 64
HW = H * W
S, CTX = 77, 768
HEADS = 8
HD = C // HEADS  # 64
GROUPS = 32
EPS = 1e-5
NCORES = 8
BPC = B // NCORES  # 2 batches per core
P = 128
NCH = HW // 512  # 8 pixel chunks of 512
KQ = C // P      # 4 chunks of 128 for C-contraction
KC = CTX // P    # 6 chunks for CTX-contraction
SCALE = HD ** (-0.5)
NU = BPC * NCH   # 16 pipeline units


def _ctx_ln(nc, pools, consts, b, ctxr, st):
    """DMA + LayerNorm(context) for batch b (vector work, no weights)."""
    small_pool = pools["small"]
    ctx_t = small_pool.tile([S, CTX], F32, tag=f"ctx{b}")
    nc.sync.dma_start(ctx_t, ctxr[b])
    lst = small_pool.tile([S, 3, 6], F32, tag=f"lst{b}")
    for i in range(3):
        nc.vector.bn_stats(lst[:, i, :], ctx_t[:, i * 256:(i + 1) * 256])
    lmv = small_pool.tile([S, 2], F32, tag=f"lmv{b}")
    nc.vector.bn_aggr(lmv, lst)
    nc.scalar.activation(lmv[:, 1:2], lmv[:, 1:2], AF.Sqrt,
                         bias=consts["eps77"], scale=1.0)
    nc.vector.reciprocal_approx_fast(out=lmv[:, 1:2], in_=lmv[:, 1:2])
    cn_t = small_pool.tile([S, CTX], F32, tag=f"cn{b}")
    nc.vector.tensor_scalar(cn_t, ctx_t, lmv[:, 0:1], lmv[:, 1:2],
                            ALU.subtract, ALU.mult)
    nc.vector.tensor_mul(cn_t, cn_t, consts["lnw_bc"])
    nc.vector.tensor_add(cn_t, cn_t, consts["lnb_bc"])
    st[b]["cn_t"] = cn_t


def _ctx_proj(nc, pools, consts, b, st):
    """cnT transpose -> kT, v_sc for batch b (PE work, needs weights)."""
    small_pool, ps_d, ps_av = pools["small"], pools["ps_d"], pools["ps_av"]
    cn_t = st[b].pop("cn_t")
    cnT = small_pool.tile([P, KC, S], BF16, tag=f"cnT{b}")
    for kc in range(KC):
        pt = ps_d.tile([P, S], F32, tag="pd")
        nc.tensor.transpose(pt, cn_t[:, kc * P:(kc + 1) * P],
                            consts["ident"][:S, :S])
        nc.vector.tensor_copy(cnT[:, kc, :], pt)

    kT = small_pool.tile([P, KQ, S], BF16, tag=f"kT{b}")
    for mo in range(KQ):
        pk = ps_av.tile([P, S], F32, tag="pav")
        for kc in range(KC):
            nc.tensor.matmul(pk, consts["kwT"][:, kc, mo * P:(mo + 1) * P],
                             cnT[:, kc, :], start=(kc == 0), stop=(kc == KC - 1))
        nc.scalar.activation(kT[:, mo, :], pk, AF.Identity,
                             bias=consts["kb"][:, mo:mo + 1], scale=1.0)

    pv = pools["ps_qk"].tile([S, C], F32, tag="pa")
    for kc in range(KC):
        nc.tensor.matmul(pv, cnT[:, kc, :], consts["vwT"][:, kc, :],
                         start=(kc == 0), stop=(kc == KC - 1))
    v_sc = small_pool.tile([S, C], BF16, tag=f"vsc{b}")
    nc.vector.tensor_add(v_sc, pv, consts["vb_bc"])

    st[b]["kT"] = kT
    st[b]["v_sc"] = v_sc


def _x_init(nc, pools, b, st):
    small_pool, xr_pool = pools["small"], pools["xr"]
    st[b]["gst"] = small_pool.tile([P, KQ, 8, 6], F32, tag=f"gst{b}",
                                   name=f"gst{b}")
    st[b]["mv_c"] = small_pool.tile([P, KQ, 2], F32, tag=f"mvc{b}",
                                    name=f"mvc{b}")
    st[b]["x_t"] = [xr_pool.tile([P, HW], BF16, tag=f"xr{b}_{co}",
                                 name=f"xr{b}_{co}")
                    for co in range(KQ)]


def _x_load_half(nc, pools, consts, b, co, h, xr, st):
    """Stream one 2048-px half of a 128-channel chunk of x for batch b:
    DMA, bn_stats per 512 slice, bias-folded bf16 residual copy."""
    x_pool = pools["x"]
    gst, mv_c, x_t = st[b]["gst"], st[b]["mv_c"], st[b]["x_t"]
    hsl = slice(h * 2048, (h + 1) * 2048)
    xf = x_pool.tile([P, 2048], F32, tag="xf")
    nc.sync.dma_start(xf, xr[b, :, co, hsl])
    for sg in range(4):
        g = h * 4 + sg
        nc.vector.bn_stats(gst[:, co, g, :],
                           xf[:, sg * 512:(sg + 1) * 512])
    nc.scalar.activation(x_t[co][:, hsl], xf, AF.Identity,
                         bias=consts["ob"][:, co:co + 1], scale=1.0)
    if h == 1:
        nc.vector.bn_aggr(mv_c[:, co, :], gst[:, co])


def _stats_combine(nc, pools, consts, b, st):
    """Combine per-partition stats into per-group scale/shift."""
    small_pool, ps_d, ps_av = pools["small"], pools["ps_d"], pools["ps_av"]
    mv_c = st[b]["mv_c"]
    t3 = small_pool.tile([P, KQ, 3], F32, tag=f"t3{b}")
    nc.vector.tensor_copy(t3[:, :, 0:2], mv_c)
    nc.vector.tensor_mul(t3[:, :, 2:3], mv_c[:, :, 0:1], mv_c[:, :, 0:1])
    pg = ps_d.tile([GROUPS // KQ, KQ * 3], F32, tag="pd")
    nc.tensor.matmul(pg, consts["ind1"],
                     t3.rearrange("p a b -> p (a b)"), start=True, stop=True)
    g_sb = small_pool.tile([GROUPS // KQ, KQ, 3], F32, tag=f"gsb{b}")
    nc.vector.tensor_copy(g_sb.rearrange("p a b -> p (a b)"), pg)
    stats2 = small_pool.tile([GROUPS // KQ, 2, KQ], F32, tag=f"st2{b}")
    nc.vector.tensor_copy(stats2[:, 0, :], g_sb[:, :, 0])
    vt = small_pool.tile([GROUPS // KQ, KQ], F32, tag=f"vt{b}")
    nc.vector.tensor_add(vt, g_sb[:, :, 1], g_sb[:, :, 2])
    m2 = small_pool.tile([GROUPS // KQ, KQ], F32, tag=f"m2{b}")
    nc.vector.tensor_mul(m2, g_sb[:, :, 0], g_sb[:, :, 0])
    nc.vector.tensor_sub(vt, vt, m2)
    nc.scalar.activation(vt, vt, AF.Sqrt, bias=consts["eps8"], scale=1.0)
    nc.vector.reciprocal_approx_fast(out=stats2[:, 1, :], in_=vt)
    pbc = ps_av.tile([P, 2 * KQ], F32, tag="pav")
    nc.tensor.matmul(pbc, consts["ind2"],
                     stats2.rearrange("p a b -> p (a b)"), start=True, stop=True)
    sbc = small_pool.tile([P, 2, KQ], F32, tag=f"sbc{b}")
    nc.vector.tensor_copy(sbc.rearrange("p a b -> p (a b)"), pbc)
    scale_c = small_pool.tile([P, KQ], F32, tag=f"scl{b}")
    shift_c = small_pool.tile([P, KQ], F32, tag=f"shf{b}")
    nc.vector.tensor_mul(scale_c, sbc[:, 1, :], consts["gnw"])
    nc.vector.tensor_mul(shift_c, sbc[:, 0, :], scale_c)
    nc.vector.tensor_sub(shift_c, consts["gnb"], shift_c)
    shift2 = small_pool.tile([P, KQ], F32, tag=f"sh2{b}")
    nc.vector.tensor_mul(shift2, consts["ob"], scale_c)
    nc.vector.tensor_sub(shift2, shift_c, shift2)
    st[b]["scale_c"] = scale_c
    st[b]["shift2"] = shift2


def _stage_a_xnt(nc, pools, consts, st, u):
    """Groupnorm apply (vector) for unit u -> xnt."""
    b, n = divmod(u, NCH)
    nsl = slice(n * 512, (n + 1) * 512)
    x_t = st[b]["x_t"]
    scale_c, shift2 = st[b]["scale_c"], st[b]["shift2"]
    xnt = pools["xn"].tile([P, KQ, 512], F8 if FP8_PROJ else BF16, tag="xnt")
    for kc in range(KQ):
        nc.vector.tensor_scalar(xnt[:, kc, :], x_t[kc][:, nsl],
                                scale_c[:, kc:kc + 1], shift2[:, kc:kc + 1],
                                ALU.mult, ALU.add)
    return xnt


def _proj(nc, pools, consts, wname, src, out_evac):
    """512x512 projection of a [128, 4, 512] tile; fp8 DoubleRow or bf16."""
    for mo in range(KQ):
        pq = pools["ps_proj"].tile([P, 512], F32, tag="pmm", name="pq")
        if FP8_PROJ:
            w8 = consts[wname + "8"]
            for j in range(2):
                nc.tensor.matmul(pq, w8[:, j, :, mo * P:(mo + 1) * P],
                                 src[:, 2 * j:2 * j + 2, :],
                                 start=(j == 0), stop=(j == 1),
                                 perf_mode=PM.DoubleRow)
        else:
            for kc in range(KQ):
                nc.tensor.matmul(pq, consts[wname][:, kc, mo * P:(mo + 1) * P],
                                 src[:, kc, :], start=(kc == 0),
                                 stop=(kc == KQ - 1))
        out_evac(mo, pq)


def _stage_a_q(nc, pools, consts, st, u, xnt):
    """q projection (PE) for unit u -> qT(u)."""
    qT = pools["q"].tile([P, KQ, 512], BF16, tag="qT")

    def evac(mo, pq):
        nc.scalar.activation(qT[:, mo, :], pq, AF.Identity,
                             bias=consts["qb"][:, mo:mo + 1], scale=1.0)

    _proj(nc, pools, consts, "qwT", xnt, evac)
    st["qT"][u] = qT


def _stage_a(nc, pools, consts, st, u):
    xnt = _stage_a_xnt(nc, pools, consts, st, u)
    _stage_a_q(nc, pools, consts, st, u, xnt)


def _stage_b(nc, pools, consts, st, u):
    """Attention for unit u: QK -> exp -> denom -> recip -> AV -> outT(u)."""
    b, n = divmod(u, NCH)
    kT, v_sc = st[b]["kT"], st[b]["v_sc"]
    qT = st["qT"].pop(u)
    ex_l = []
    for co in range(KQ):
        pa = pools["ps_qk"].tile([S, 2, 512], F32, tag="pa")
        nc.tensor.matmul(pa[:, 0, :], kT[0:HD, co, :], qT[0:HD, co, :],
                         start=True, stop=True, tile_position=(0, 0))
        nc.tensor.matmul(pa[:, 1, :], kT[HD:P, co, :], qT[HD:P, co, :],
                         start=True, stop=True, tile_position=(64, 0))
        ex = pools["exp"].tile([S, 2, 512], BF16, tag="ex")
        nc.scalar.activation(ex, pa, AF.Exp, scale=SCALE)
        ex_l.append(ex)
    outT = pools["o"].tile([P, KQ, 512], F8 if FP8_PROJ else BF16, tag="outT")
    for co in range(KQ):
        ex = ex_l[co]
        pd = pools["ps_d"].tile([P, 512], F32, tag="pd")
        nc.tensor.matmul(pd[0:HD, :], consts["ones77"], ex[:, 0, :],
                         start=True, stop=True, tile_position=(0, 0))
        nc.tensor.matmul(pd[HD:P, :], consts["ones77"], ex[:, 1, :],
                         start=True, stop=True, tile_position=(0, 64))
        pav = pools["ps_av"].tile([P, 512], F32, tag="pav")
        h0, h1 = 2 * co, 2 * co + 1
        nc.tensor.matmul(pav[0:HD, :], v_sc[:, h0 * HD:(h0 + 1) * HD],
                         ex[:, 0, :], start=True, stop=True,
                         tile_position=(0, 0))
        nc.tensor.matmul(pav[HD:P, :], v_sc[:, h1 * HD:(h1 + 1) * HD],
                         ex[:, 1, :], start=True, stop=True,
                         tile_position=(0, 64))
        rc = pools["rc"].tile([P, 512], F32, tag="rc")
        nc.vector.reciprocal_approx_fast(out=rc, in_=pd)
        nc.vector.tensor_mul(outT[:, co, :], pav, rc)
    st["outT"][u] = outT


def _stage_c(nc, pools, consts, st, u, outr):
    """Out projection + residual + store for unit u."""
    b, n = divmod(u, NCH)
    nsl = slice(n * 512, (n + 1) * 512)
    x_t = st[b]["x_t"]
    outT = st["outT"].pop(u)

    def evac(mo, po):
        fin = pools["fin"].tile([P, 512], F32, tag="fin", name="fin")
        nc.vector.tensor_add(fin, po, x_t[mo][:, nsl])
        nc.sync.dma_start(outr[b, :, mo, nsl], fin)

    _proj(nc, pools, consts, "owT", outT, evac)


def build_nc(reps=1, loop_reps=0):
    nc = bacc.Bacc()

    x = nc.dram_tensor("x", [BPC, C, HW], F32, kind="ExternalInput")
    ctx_in = nc.dram_tensor("ctx", [BPC, S, CTX], F32, kind="ExternalInput")
    # weights pre-arranged host-side: [kp, ko, out]
    kwT = nc.dram_tensor("kwT", [P, KC, C], BF16, kind="ExternalInput")
    vwT = nc.dram_tensor("vwT", [P, KC, C], BF16, kind="ExternalInput")
    if FP8_PROJ:
        # [kp, j, plane, out] for DoubleRow: input channel (2j+plane)*128+kp
        qw8 = nc.dram_tensor("qw8", [P, 2, 2, C], F8, kind="ExternalInput")
        ow8 = nc.dram_tensor("ow8", [P, 2, 2, C], F8, kind="ExternalInput")
    else:
        qwT = nc.dram_tensor("qwT", [P, KQ, C], BF16, kind="ExternalInput")
        owT = nc.dram_tensor("owT", [P, KQ, C], BF16, kind="ExternalInput")
    qb = nc.dram_tensor("qb", [C], F32, kind="ExternalInput")
    kb = nc.dram_tensor("kb", [C], F32, kind="ExternalInput")
    vb = nc.dram_tensor("vb", [C], F32, kind="ExternalInput")
    ob = nc.dram_tensor("ob", [C], F32, kind="ExternalInput")
    gnw = nc.dram_tensor("gnw", [C], F32, kind="ExternalInput")
    gnb = nc.dram_tensor("gnb", [C], F32, kind="ExternalInput")
    lnw = nc.dram_tensor("lnw", [CTX], F32, kind="ExternalInput")
    lnb = nc.dram_tensor("lnb", [CTX], F32, kind="ExternalInput")
    ident = nc.dram_tensor("ident", [P, P], F32, kind="ExternalInput")
    ones77 = nc.dram_tensor("ones77", [S, HD], BF16, kind="ExternalInput")
    ind1 = nc.dram_tensor("ind1", [P, GROUPS // KQ], F32, kind="ExternalInput")
    ind2 = nc.dram_tensor("ind2", [GROUPS // KQ, P], F32, kind="ExternalInput")
    out = nc.dram_tensor("out", [BPC, C, HW], F32, kind="ExternalOutput")

    xr = x[:].rearrange("b (co p) hw -> b p co hw", p=P)
    ctxr = ctx_in[:]
    outr = out[:].rearrange("b (co p) hw -> b p co hw", p=P)

    with tile.TileContext(nc) as tc:
        with (
            tc.tile_pool(name="singles", bufs=1) as singles,
            tc.tile_pool(name="xp", bufs=3) as x_pool,
            tc.tile_pool(name="xr", bufs=1) as xr_pool,
            tc.tile_pool(name="xnp", bufs=3) as xn_pool,
            tc.tile_pool(name="small", bufs=1) as small_pool,
            tc.tile_pool(name="qp", bufs=3) as q_pool,
            tc.tile_pool(name="op", bufs=2) as o_pool,
            tc.tile_pool(name="expp", bufs=5) as exp_pool,
            tc.tile_pool(name="rcp", bufs=2) as rc_pool,
            tc.tile_pool(name="finp", bufs=4) as fin_pool,
            tc.tile_pool(name="ps_proj", bufs=2, space="PSUM") as ps_proj,
            tc.tile_pool(name="ps_qk", bufs=2, space="PSUM") as ps_qk,
            tc.tile_pool(name="ps_d", bufs=1, space="PSUM") as ps_d,
            tc.tile_pool(name="ps_av", bufs=1, space="PSUM") as ps_av,
        ):
            pools = {
                "x": x_pool, "xr": xr_pool, "xn": xn_pool, "small": small_pool,
                "q": q_pool, "o": o_pool, "exp": exp_pool, "rc": rc_pool,
                "fin": fin_pool, "ps_proj": ps_proj, "ps_qk": ps_qk,
                "ps_d": ps_d, "ps_av": ps_av,
            }
            consts = {}

            def _small_consts():
                for name, src in (("qb", qb), ("kb", kb), ("ob", ob),
                                  ("gnw", gnw), ("gnb", gnb)):
                    t = singles.tile([P, KQ], F32, tag=name)
                    nc.sync.dma_start(t, src[:].rearrange("(a p) -> p a", p=P))
                    consts[name] = t
                t = singles.tile([P, P], F32, tag="ident")
                nc.sync.dma_start(t, ident[:])
                consts["ident"] = t
                t = singles.tile([S, HD], BF16, tag="ones77")
                nc.sync.dma_start(t, ones77[:])
                consts["ones77"] = t
                t = singles.tile([P, GROUPS // KQ], F32, tag="ind1")
                nc.sync.dma_start(t, ind1[:])
                consts["ind1"] = t
                t = singles.tile([GROUPS // KQ, P], F32, tag="ind2")
                nc.sync.dma_start(t, ind2[:])
                consts["ind2"] = t
                t = singles.tile([S, C], F32, tag="vb_bc")
                nc.gpsimd.dma_start(out=t, in_=vb[None, :].to_broadcast([S, C]))
                consts["vb_bc"] = t
                for name, src in (("lnw_bc", lnw), ("lnb_bc", lnb)):
                    t = singles.tile([S, CTX], F32, tag=name)
                    nc.gpsimd.dma_start(out=t,
                                        in_=src[None, :].to_broadcast([S, CTX]))
                    consts[name] = t
                t = singles.tile([S, 1], F32, tag="eps77")
                nc.vector.memset(t, EPS)
                consts["eps77"] = t
                t = singles.tile([GROUPS // KQ, 1], F32, tag="eps8")
                nc.vector.memset(t, EPS)
                consts["eps8"] = t

            def _weights():
                for name, src, ko in (("kwT", kwT, KC), ("vwT", vwT, KC)):
                    t = singles.tile([P, ko, C], BF16, tag=name, name=name)
                    nc.sync.dma_start(t, src[:])
                    consts[name] = t
                if FP8_PROJ:
                    for name, src in (("qwT8", qw8), ("owT8", ow8)):
                        t = singles.tile([P, 2, 2, C], F8, tag=name, name=name)
                        nc.sync.dma_start(t, src[:])
                        consts[name] = t
                else:
                    for name, src in (("qwT", qwT), ("owT", owT)):
                        t = singles.tile([P, KQ, C], BF16, tag=name, name=name)
                        nc.sync.dma_start(t, src[:])
                        consts[name] = t

            def build_once():
                st = {0: {}, 1: {}, "qT": {}, "outT": {}}
                # tiny DMAs first so ctx + consts land before the x flood
                _small_consts()
                for b in range(BPC):
                    _ctx_ln(nc, pools, consts, b, ctxr, st)
                _weights()
                # batch-0 x stream (DMA emission before ctx-side PE work so
                # the rings prioritize it; the PE work depends only on ctx)
                _x_init(nc, pools, 0, st)
                for co in range(KQ):
                    for h in range(2):
                        _x_load_half(nc, pools, consts, 0, co, h, xr, st)
                for b in range(BPC):
                    _ctx_proj(nc, pools, consts, b, st)
                _stats_combine(nc, pools, consts, 0, st)
                _x_init(nc, pools, 1, st)
                # skewed pipeline: A two ahead, C at current; batch-1 x
                # stream spread per-half over the first iterations
                b1_sched = {0: [(0, 0)], 1: [(0, 1)], 2: [(1, 0)],
                            3: [(1, 1)], 4: [(2, 0), (2, 1)],
                            5: [(3, 0), (3, 1)]}
                _stage_a(nc, pools, consts, st, 0)
                _stage_a(nc, pools, consts, st, 1)
                for u in range(NU):
                    xnt_next = (_stage_a_xnt(nc, pools, consts, st, u + 2)
                                if u + 2 < NU else None)
                    _stage_b(nc, pools, consts, st, u)
                    if xnt_next is not None:
                        _stage_a_q(nc, pools, consts, st, u + 2, xnt_next)
                    for co, h in b1_sched.get(u, ()):
                        _x_load_half(nc, pools, consts, 1, co, h, xr, st)
                    if u == 5:
                        _stats_combine(nc, pools, consts, 1, st)
                    _stage_c(nc, pools, consts, st, u, outr)

            if loop_reps:
                with tc.For_i(0, loop_reps, 1):
                    build_once()
            else:
                for _rep in range(reps):
                    build_once()

    nc.finalize()
    return nc


_NC_CACHE = None


def _get_nc():
    global _NC_CACHE
    if _NC_CACHE is None:
        _NC_CACHE = build_nc()
    return _NC_CACHE


def _host_consts():
    bf = ml_dtypes.bfloat16
    g = GROUPS // KQ  # 8 groups per 128-channel chunk
    ind1 = np.zeros((P, g), np.float32)
    for p in range(P):
        ind1[p, p // 16] = 1.0 / 16.0
    ind2 = np.zeros((g, P), np.float32)
    for p in range(P):
        ind2[p // 16, p] = 1.0
    return {
        "ident": np.eye(P, dtype=np.float32),
        "ones77": np.ones((S, HD), dtype=bf),
        "ind1": ind1,
        "ind2": ind2,
    }


def _w_arrange(w, ko):
    """[out, in] weight -> [kp, ko, out] bf16 with contiguous 4KB lines."""
    bf = ml_dtypes.bfloat16
    wT = np.ascontiguousarray(np.asarray(w, np.float32).T)  # [in, out]
    return np.ascontiguousarray(
        wT.reshape(ko, P, wT.shape[1]).transpose(1, 0, 2)).astype(bf)


def _w8_arrange(w):
    """[out, in] weight -> [kp, j, plane, out] fp8e4m3 for DoubleRow."""
    f8 = ml_dtypes.float8_e4m3fn
    wT = np.ascontiguousarray(np.asarray(w, np.float32).T)  # [in, out]
    return np.ascontiguousarray(
        wT.reshape(2, 2, P, wT.shape[1]).transpose(2, 0, 1, 3)).astype(f8)


def make_in_maps(x, context, gn_w, gn_b, ln_w, ln_b, q_w, q_b, k_w, k_b,
                 v_w, v_b, out_w, out_b):
    x = np.asarray(x, np.float32).reshape(B, C, HW)
    context = np.ascontiguousarray(np.asarray(context, np.float32))
    if FP8_PROJ:
        wmaps = {"qw8": _w8_arrange(q_w), "ow8": _w8_arrange(out_w)}
    else:
        wmaps = {"qwT": _w_arrange(q_w, KQ), "owT": _w_arrange(out_w, KQ)}
    shared = {
        **wmaps,
        "kwT": _w_arrange(k_w, KC),
        "vwT": _w_arrange(v_w, KC),
        "qb": np.asarray(q_b, np.float32),
        "kb": np.asarray(k_b, np.float32),
        "vb": np.asarray(v_b, np.float32),
        "ob": np.asarray(out_b, np.float32),
        "gnw": np.asarray(gn_w, np.float32),
        "gnb": np.asarray(gn_b, np.float32),
        "lnw": np.asarray(ln_w, np.float32),
        "lnb": np.asarray(ln_b, np.float32),
        **_host_consts(),
    }
    in_maps = []
    for i in range(NCORES):
        m = dict(shared)
        m["x"] = np.ascontiguousarray(x[i * BPC:(i + 1) * BPC])
        m["ctx"] = np.ascontiguousarray(context[i * BPC:(i + 1) * BPC])
        in_maps.append(m)
    return in_maps


def kernel(x, context, gn_w, gn_b, ln_w, ln_b, q_w, q_b, k_w, k_b,
           v_w, v_b, out_w, out_b):
    in_maps = make_in_maps(x, context, gn_w, gn_b, ln_w, ln_b, q_w, q_b,
                           k_w, k_b, v_w, v_b, out_w, out_b)
    nc = _get_nc()
    res = run_bass_kernel_spmd(nc, in_maps, core_ids=list(range(NCORES)))
    outs = [r["out"] for r in res.results]
    return np.concatenate(outs, axis=0).reshape(B, C, H, W)


if __name__ == "__main__":
    rng = np.random.default_rng(0)
    inputs = {
        "x": rng.standard_normal((B, C, H, W), np.float32),
        "context": rng.standard_normal((B, S, CTX), np.float32),
        "gn_w": np.ones(C, np.float32), "gn_b": np.zeros(C, np.float32),
        "ln_w": np.ones(CTX, np.float32), "ln_b": np.zeros(CTX, np.float32),
        "q_w": rng.standard_normal((C, C), np.float32) * 0.02,
        "q_b": np.zeros(C, np.float32),
        "k_w": rng.standard_normal((C, CTX), np.float32) * 0.02,
        "k_b": np.zeros(C, np.float32),
        "v_w": rng.standard_normal((C, CTX), np.float32) * 0.02,
        "v_b": np.zeros(C, np.float32),
        "out_w": rng.standard_normal((C, C), np.float32) * 0.02,
        "out_b": np.zeros(C, np.float32),
    }
    out = kernel(**inputs)
    print(out.shape, out.dtype)
